# revision 2
# baseline (speedup 1.0000x reference)
"""GCN (2-layer + MLP head) on 8 Trainium2 NeuronCores — v3.

Same architecture as the v1 baseline (node-sharded, AllGather table,
dma_gather/dma_scatter_add edge aggregation), with the serialization
fixed.  The v1 trace showed Pool (descriptor-gen, 13.4ms busy) and the
16 DMA engines (14.3ms busy) running back-to-back, not overlapped,
because each scatter's Pool instruction waited for its gather's DMA
completion, and consecutive scatters serialized on the single agg
tensor.  v3:

 1. Variable-size occurrence-round calls (rounds padded to 128 tokens,
    not 1024) — removes ~10%% padding descriptors and most per-call
    fixed costs; calls capped at 4096 tokens (SWDGE ring capacity).
 2. Software-pipelined emission with a lag between gather k and
    scatter k, so the scatter's wait on the gather DMA is pre-satisfied
    when it reaches the Pool engine.
 3. Scatters round-robin over 4 agg DRAM tensors so consecutive
    scatter calls have no WAW dependency; pointwise sums the parts.
 4. Index tiles streamed from DRAM per call instead of resident in
    SBUF (frees ~124KB/partition for the deeper pipeline).
"""
import numpy as np

import concourse.bacc as bacc
import concourse.mybir as mybir
from concourse.tile import TileContext
from concourse.bass_utils import run_bass_kernel_spmd

N = 100000
NS_RAW = 12500          # real nodes per core
NS = 12544              # padded (98 * 128)
NTILE = NS // 128       # 98
N8 = NS * 8             # padded table rows
CHUNK = N8 // 4         # 25088 (< int16 max)
IN_CH, HID, HID2, OUT = 256, 64, 32, 2
CMAX = 1024             # max tokens per gather/scatter call
TRASH = NS - 1          # trash dst row for padding tokens
NAGG = 4                # round-robin agg tensors
LAG = 3                 # scatter k trails gather k by LAG calls

_compiled = {}          # schedule-signature -> nc
_SIM_NO_CC = False      # replace AllGather with local copies (single-core sim)


def _build_schedule(src, dst):
    """Token streams per core: grouped by src-chunk; within a chunk ordered
    by (occurrence-round, dst).  Rounds padded to multiples of 128 tokens
    (cross-core max), then cut into calls of at most CMAX tokens."""
    E = src.shape[0]
    core = dst // NS_RAW
    dstl = (dst % NS_RAW).astype(np.int64)
    tidx = (src // NS_RAW) * NS + (src % NS_RAW)
    chunk = tidx // CHUNK
    srcl = (tidx % CHUNK).astype(np.int64)

    key = (core * 4 + chunk) * NS + dstl
    order = np.argsort(key, kind="stable")
    ks = key[order]
    first = np.r_[True, ks[1:] != ks[:-1]]
    gs = np.where(first, np.arange(E), 0)
    np.maximum.accumulate(gs, out=gs)
    rank = np.arange(E) - gs
    rank_e = np.empty(E, np.int64)
    rank_e[order] = rank

    # round sizes per (core, chunk, round) -> shared (max over cores)
    nr = int(rank_e.max()) + 1
    cnt = np.zeros((8, 4, nr), np.int64)
    np.add.at(cnt, (core, chunk, rank_e), 1)
    shared = cnt.max(axis=0)                      # [4, nr]
    shared = (shared + 127) // 128 * 128          # pad rounds to 128

    # stream layout: chunk-major, then rounds
    round_base = np.zeros((4, nr), np.int64)
    tok = 0
    for k in range(4):
        for r in range(nr):
            round_base[k, r] = tok
            tok += int(shared[k, r])
    TOK = tok

    # position of each edge: round_base + offset within (core,chunk,round)
    # offset = rank among edges of same (core,chunk,round), ordered by dst
    key2 = ((core * 4 + chunk) * nr + rank_e)
    order2 = np.argsort(key2 * NS + dstl, kind="stable")
    k2s = key2[order2]
    first2 = np.r_[True, k2s[1:] != k2s[:-1]]
    gs2 = np.where(first2, np.arange(E), 0)
    np.maximum.accumulate(gs2, out=gs2)
    off2 = np.arange(E) - gs2
    off_e = np.empty(E, np.int64)
    off_e[order2] = off2

    tpos = round_base[chunk, rank_e] + off_e

    src16 = np.zeros((8, TOK), np.int16)
    dst16 = np.full((8, TOK), TRASH, np.int16)
    for c in range(8):
        m = core == c
        src16[c, tpos[m]] = srcl[m].astype(np.int16)
        dst16[c, tpos[m]] = dstl[m].astype(np.int16)

    # calls: split rounds into <=CMAX pieces (multiples of 128)
    calls = []
    for k in range(4):
        for r in range(nr):
            sz = int(shared[k, r])
            base = int(round_base[k, r])
            off = 0
            while off < sz:
                n = min(CMAX, sz - off)
                calls.append((base + off, n, k))
                off += n
    return src16, dst16, calls, TOK


def _wrap16(a):
    w = a.reshape(-1, 16).T.copy()
    return np.tile(w, (8, 1))


def _build_program(calls, TOK):
    nc = bacc.Bacc(None, target_bir_lowering=False,
                   dynamic_dma_scratch_size=49152)
    dt = mybir.dt
    P = nc.declare_dram_parameter
    xT = P("xT", [IN_CH, NS], dt.float32, isOutput=False)
    w1p = P("w1p", [128, 128], dt.float32, isOutput=False)
    w2 = P("w2", [HID, HID], dt.float32, isOutput=False)
    wh1 = P("wh1", [HID, HID2], dt.float32, isOutput=False)
    wh2 = P("wh2", [HID2, OUT], dt.float32, isOutput=False)
    b1f = P("b1f", [128, HID], dt.float32, isOutput=False)
    b2f = P("b2f", [128, HID], dt.float32, isOutput=False)
    bh1 = P("bh1", [HID2, 1], dt.float32, isOutput=False)
    bh2 = P("bh2", [OUT, 1], dt.float32, isOutput=False)
    dinvP = P("dinvP", [128, NTILE], dt.float32, isOutput=False)
    src16 = P("src16", [128, TOK // 16], dt.int16, isOutput=False)
    dst16 = P("dst16", [128, TOK // 16], dt.int16, isOutput=False)
    outT = P("outT", [OUT, NS], dt.float32, isOutput=True)

    slice_d = [nc.dram_tensor(f"slice{l}", [NS, HID], dt.float32) for l in (1, 2)]
    table_d = [nc.dram_tensor(f"table{l}", [N8, HID], dt.float32) for l in (1, 2)]
    agg_d = [[nc.dram_tensor(f"agg{l}_{a}", [NS, HID], dt.float32)
              for a in range(NAGG)] for l in (0, 1)]

    from concourse.masks import make_identity
    relu = mybir.ActivationFunctionType.Relu

    with TileContext(nc) as tc:
        with tc.tile_pool(name="const", bufs=1) as cp, \
             tc.tile_pool(name="work", bufs=3) as wp, \
             tc.tile_pool(name="tok", bufs=LAG + 3) as tp, \
             tc.tile_pool(name="idx", bufs=LAG + 3) as ip, \
             tc.tile_pool(name="hT", bufs=1) as hp, \
             tc.tile_pool(name="ps", bufs=2, space="PSUM") as pp:
            w1sb = cp.tile([128, 128], dt.float32)
            nc.sync.dma_start(out=w1sb[:], in_=w1p[:])
            w2sb = cp.tile([HID, HID], dt.float32)
            nc.sync.dma_start(out=w2sb[:], in_=w2[:])
            wh1sb = cp.tile([HID, HID2], dt.float32)
            nc.sync.dma_start(out=wh1sb[:], in_=wh1[:])
            wh2sb = cp.tile([HID2, OUT], dt.float32)
            nc.sync.dma_start(out=wh2sb[:], in_=wh2[:])
            b1sb = cp.tile([128, HID], dt.float32)
            nc.sync.dma_start(out=b1sb[:], in_=b1f[:])
            b2sb = cp.tile([128, HID], dt.float32)
            nc.sync.dma_start(out=b2sb[:], in_=b2f[:])
            bh1sb = cp.tile([HID2, 1], dt.float32)
            nc.sync.dma_start(out=bh1sb[:], in_=bh1[:])
            bh2sb = cp.tile([OUT, 1], dt.float32)
            nc.sync.dma_start(out=bh2sb[:], in_=bh2[:])
            dsb = cp.tile([128, NTILE], dt.float32)
            nc.sync.dma_start(out=dsb[:], in_=dinvP[:])
            ident = cp.tile([128, 128], dt.float32)
            make_identity(nc, ident[:])
            zt = cp.tile([128, 512], dt.float32)
            nc.gpsimd.memset(zt[:], 0.0)

            def zero_aggs(aggs):
                zt3 = zt[:].rearrange("p (a d) -> p a d", d=HID)
                for agg in aggs:
                    agg3 = agg[:].rearrange("(a p) d -> p a d", p=128)
                    a0 = 0
                    while a0 < NTILE:
                        an = min(8, NTILE - a0)
                        nc.sync.dma_start(out=agg3[:, a0:a0 + an, :],
                                          in_=zt3[:, 0:an, :])
                        a0 += an

            def aggregate(table, aggs):
                pend = []

                def do_scatter(gt, di, ntok, ai):
                    g3 = gt[:, 0:(ntok // 128) * HID].rearrange(
                        "p (c d) -> p c d", d=HID)
                    nc.gpsimd.dma_scatter_add(
                        aggs[ai][:], g3, di[:, 0:ntok // 16], ntok, ntok, HID)

                for i, (t0, ntok, k) in enumerate(calls):
                    o = t0 // 16
                    si = ip.tile([128, CMAX // 16], dt.int16, tag="si")
                    nc.sync.dma_start(out=si[:, 0:ntok // 16],
                                      in_=src16[:, o:o + ntok // 16])
                    di = ip.tile([128, CMAX // 16], dt.int16, tag="di")
                    nc.sync.dma_start(out=di[:, 0:ntok // 16],
                                      in_=dst16[:, o:o + ntok // 16])
                    gt = tp.tile([128, (CMAX // 128) * HID], dt.float32, tag="gt")
                    g3 = gt[:, 0:(ntok // 128) * HID].rearrange(
                        "p (c d) -> p c d", d=HID)
                    nc.gpsimd.dma_gather(
                        g3, table[k * CHUNK:(k + 1) * CHUNK, :],
                        si[:, 0:ntok // 16], ntok, ntok, HID)
                    pend.append((gt, di, ntok, i % NAGG))
                    if len(pend) > LAG:
                        do_scatter(*pend.pop(0))
                while pend:
                    do_scatter(*pend.pop(0))

            # ---- layer 1 GEMM ----
            for m in range(NTILE):
                mc = slice(m * 128, (m + 1) * 128)
                xa = wp.tile([128, 128], dt.float32, tag="xa")
                nc.sync.dma_start(out=xa[:], in_=xT[0:128, mc])
                xb = wp.tile([128, 128], dt.float32, tag="xb")
                nc.sync.dma_start(out=xb[:], in_=xT[128:256, mc])
                ps = pp.tile([128, HID], dt.float32, tag="ps")
                nc.tensor.matmul(ps[:], xa[:], w1sb[:, 0:HID], start=True, stop=False)
                nc.tensor.matmul(ps[:], xb[:], w1sb[:, HID:128], start=False, stop=True)
                hw = wp.tile([128, HID], dt.float32, tag="hw")
                nc.vector.tensor_scalar_mul(hw[:], ps[:], dsb[:, m:m + 1])
                nc.sync.dma_start(out=slice_d[0][mc, :], in_=hw[:])
            if _SIM_NO_CC:
                for cc in range(8):
                    nc.sync.dma_start(out=table_d[0][cc * NS:(cc + 1) * NS, :],
                                      in_=slice_d[0][:])
            else:
                nc.gpsimd.collective_compute(
                    "AllGather", mybir.AluOpType.bypass,
                    replica_groups=[list(range(8))],
                    ins=[slice_d[0][:]], outs=[table_d[0][:]])
            zero_aggs(agg_d[0])
            aggregate(table_d[0], agg_d[0])

            # ---- layer-1 pointwise + transpose; layer-2 GEMM (halved hT) ----
            HTN = NTILE // 2
            for half in range(2):
                mlo = half * HTN
                hT = hp.tile([HID, HTN * 128], dt.float32, tag="hT")
                for mi in range(HTN):
                    m = mlo + mi
                    mc = slice(m * 128, (m + 1) * 128)
                    s = wp.tile([128, HID], dt.float32, tag="s")
                    at = wp.tile([128, HID], dt.float32, tag="at")
                    nc.sync.dma_start(out=at[:], in_=agg_d[0][0][mc, :])
                    st = wp.tile([128, HID], dt.float32, tag="st")
                    nc.sync.dma_start(out=st[:], in_=slice_d[0][mc, :])
                    nc.vector.tensor_add(s[:], at[:], st[:])
                    for a in range(1, NAGG):
                        at2 = wp.tile([128, HID], dt.float32, tag="at")
                        nc.sync.dma_start(out=at2[:], in_=agg_d[0][a][mc, :])
                        nc.vector.tensor_add(s[:], s[:], at2[:])
                    nc.vector.tensor_scalar_mul(s[:], s[:], dsb[:, m:m + 1])
                    nc.vector.tensor_add(s[:], s[:], b1sb[:])
                    h = wp.tile([128, HID], dt.float32, tag="h")
                    nc.scalar.activation(h[:], s[:], relu)
                    pt = pp.tile([HID, 128], dt.float32, tag="pt")
                    nc.tensor.transpose(pt[:], h[:], ident[:])
                    nc.vector.tensor_copy(hT[:, mi * 128:(mi + 1) * 128], pt[:])
                for mi in range(HTN):
                    m = mlo + mi
                    mc = slice(m * 128, (m + 1) * 128)
                    ps = pp.tile([128, HID], dt.float32, tag="ps")
                    nc.tensor.matmul(ps[:], hT[:, mi * 128:(mi + 1) * 128], w2sb[:],
                                     start=True, stop=True)
                    hw = wp.tile([128, HID], dt.float32, tag="hw")
                    nc.vector.tensor_scalar_mul(hw[:], ps[:], dsb[:, m:m + 1])
                    nc.sync.dma_start(out=slice_d[1][mc, :], in_=hw[:])
            if _SIM_NO_CC:
                for cc in range(8):
                    nc.sync.dma_start(out=table_d[1][cc * NS:(cc + 1) * NS, :],
                                      in_=slice_d[1][:])
            else:
                nc.gpsimd.collective_compute(
                    "AllGather", mybir.AluOpType.bypass,
                    replica_groups=[list(range(8))],
                    ins=[slice_d[1][:]], outs=[table_d[1][:]])
            zero_aggs(agg_d[1])
            aggregate(table_d[1], agg_d[1])

            # ---- layer-2 pointwise + transpose; head (halved hT) ----
            for half in range(2):
                mlo = half * HTN
                hT = hp.tile([HID, HTN * 128], dt.float32, tag="hT")
                for mi in range(HTN):
                    m = mlo + mi
                    mc = slice(m * 128, (m + 1) * 128)
                    s = wp.tile([128, HID], dt.float32, tag="s")
                    at = wp.tile([128, HID], dt.float32, tag="at")
                    nc.sync.dma_start(out=at[:], in_=agg_d[1][0][mc, :])
                    st = wp.tile([128, HID], dt.float32, tag="st")
                    nc.sync.dma_start(out=st[:], in_=slice_d[1][mc, :])
                    nc.vector.tensor_add(s[:], at[:], st[:])
                    for a in range(1, NAGG):
                        at2 = wp.tile([128, HID], dt.float32, tag="at")
                        nc.sync.dma_start(out=at2[:], in_=agg_d[1][a][mc, :])
                        nc.vector.tensor_add(s[:], s[:], at2[:])
                    nc.vector.tensor_scalar_mul(s[:], s[:], dsb[:, m:m + 1])
                    nc.vector.tensor_add(s[:], s[:], b2sb[:])
                    h = wp.tile([128, HID], dt.float32, tag="h")
                    nc.scalar.activation(h[:], s[:], relu)
                    pt = pp.tile([HID, 128], dt.float32, tag="pt")
                    nc.tensor.transpose(pt[:], h[:], ident[:])
                    nc.vector.tensor_copy(hT[:, mi * 128:(mi + 1) * 128], pt[:])
                for n0 in range(0, HTN * 128, 448):
                    ncol = slice(n0, n0 + 448)
                    gcol = slice(mlo * 128 + n0, mlo * 128 + n0 + 448)
                    pz = pp.tile([HID2, 448], dt.float32, tag="pz")
                    nc.tensor.matmul(pz[:], wh1sb[:], hT[:, ncol], start=True, stop=True)
                    zb = wp.tile([HID2, 448], dt.float32, tag="zb")
                    nc.scalar.activation(zb[:], pz[:], relu, bias=bh1sb[:])
                    po = pp.tile([OUT, 448], dt.float32, tag="po")
                    nc.tensor.matmul(po[:], wh2sb[:], zb[:], start=True, stop=True)
                    ob = wp.tile([OUT, 448], dt.float32, tag="ob")
                    nc.vector.tensor_scalar_add(ob[:], po[:], bh2sb[:])
                    nc.sync.dma_start(out=outT[:, gcol], in_=ob[:])

    nc.finalize()
    return nc


def kernel(x, edge_index, W1, b1, W2, b2, Wh1, bh1, Wh2, bh2, _trace=False):
    x = np.asarray(x, np.float32)
    src = np.asarray(edge_index[0], np.int64)
    dst = np.asarray(edge_index[1], np.int64)

    src16, dst16, calls, TOK = _build_schedule(src, dst)
    sig = (TOK, tuple(calls))
    if sig not in _compiled:
        _compiled[sig] = _build_program(calls, TOK)
    nc = _compiled[sig]

    deg = np.bincount(dst, minlength=N).astype(np.float64) + 1.0
    dinv = (1.0 / np.sqrt(deg)).astype(np.float32)

    W1 = np.asarray(W1, np.float32)
    w1p = np.concatenate([W1[:128], W1[128:]], axis=1)  # [128, 128]
    b1f = np.tile(np.asarray(b1, np.float32)[None, :], (128, 1))
    b2f = np.tile(np.asarray(b2, np.float32)[None, :], (128, 1))
    bh1c = np.asarray(bh1, np.float32)[:, None]
    bh2c = np.asarray(bh2, np.float32)[:, None]

    in_maps = []
    for c in range(8):
        xs = np.zeros((NS, IN_CH), np.float32)
        xs[:NS_RAW] = x[c * NS_RAW:(c + 1) * NS_RAW]
        dv = np.ones(NS, np.float32)
        dv[:NS_RAW] = dinv[c * NS_RAW:(c + 1) * NS_RAW]
        in_maps.append({
            "xT": np.ascontiguousarray(xs.T),
            "w1p": np.ascontiguousarray(w1p),
            "w2": np.asarray(W2, np.float32),
            "wh1": np.asarray(Wh1, np.float32),
            "wh2": np.asarray(Wh2, np.float32),
            "b1f": b1f, "b2f": b2f, "bh1": bh1c, "bh2": bh2c,
            "dinvP": np.ascontiguousarray(dv.reshape(NTILE, 128).T),
            "src16": _wrap16(src16[c]),
            "dst16": _wrap16(dst16[c]),
        })

    res = run_bass_kernel_spmd(nc, in_maps, list(range(8)), trace=_trace)
    out = np.empty((N, OUT), np.float32)
    for c in range(8):
        out[c * NS_RAW:(c + 1) * NS_RAW] = res.results[c]["outT"].T[:NS_RAW]
    if _trace:
        kernel.last_results = res
    return out
